# revision 1
# baseline (speedup 1.0000x reference)
"""Trainium2 Bass kernel for nn_Encoder_Decoder_fc (encoder LSTM -> decoder LSTMCell + Linear).

Strategy: data-parallel over batch (B=256 -> 32 per core on 8 cores), weights replicated.
Per core, per time step, gates are computed in a "folded" layout:
  PSUM G[32*j + b, c*128 + f] = gate_c[b, 128*j + f]   (j: H-slice 0..3, b: batch 0..31,
                                                         c: 0=i,1=f,2=o,3=g, f: 0..127)
so activations/elementwise use all 128 partitions. The recurrent matmul uses the small
h^T chunks as the PE stationary operand (cheap LDWEIGHTS) with the big Whh^T streaming,
packed 4-wide with tile_position column tiling. One PE transpose of h per step produces
the next step's stationary chunks. Decoder h^T slabs accumulate in an SBUF ring; the
output Linear is applied as batched matmuls once per 16 steps.
"""

import sys

sys.path.insert(0, "/opt/trn_rl_repo")

from contextlib import ExitStack

import ml_dtypes
import numpy as np

import concourse.bass as bass
import concourse.mybir as mybir
import concourse.tile as tile
from concourse import bacc
from concourse.bass_utils import run_bass_kernel_spmd
from concourse.masks import make_identity

P = 128
H = 512
B = 256
T = 512
N_CORES = 8
BL = B // N_CORES  # 32 batch per core
KC = H // P  # 4 contraction chunks
JC = 4  # partition groups (H-slices)
GF = 4 * P  # 512: gates free size per group [i|f|o|g]
RING = 32  # h^T ring slabs (2 windows of 16)
WIN = 16  # ys window size (steps)

F32 = mybir.dt.float32
F32R = mybir.dt.float32r
BF16 = mybir.dt.bfloat16
AF = mybir.ActivationFunctionType
_MMDT = {"bf16": BF16, "fp32": F32, "fp32r": F32R}

# fold order within free dim: i, f, o, g ; torch row offsets: i=0, f=512, g=1024, o=1536
_CBASE = (0 * H, 1 * H, 3 * H, 2 * H)  # i, f, o, g


def _perm_fold() -> np.ndarray:
    """perm[j*GF + c*P + f] = torch row index of Whh/[Wih/bias] for folded column."""
    idx = np.empty(4 * H, dtype=np.int64)
    for j in range(JC):
        for c in range(4):
            base = j * GF + c * P
            idx[base : base + P] = _CBASE[c] + j * P + np.arange(P)
    return idx


def _step(
    nc,
    pools,
    consts,
    t_abs,
    prev_slab,
    cur_slab,
    sW,
    sWU,
    c_tile,
    first_step,
    skip_main,
    mmdt,
    tap_A=None,
):
    """One LSTM step. Reads h^T from ring slab prev_slab, writes new h^T to cur_slab."""
    gpool, tpool, apool, spool, ring, sXT, ident = (
        pools["g"],
        pools["t"],
        pools["a"],
        pools["s"],
        consts["ring"],
        consts["XT"],
        consts["ident"],
    )

    G = gpool.tile([P, GF], F32, tag="G")
    # input + bias term: G[32j+b, n] = x_t[b] * Wih[perm_n] + bias[perm_n]  (PSUM init)
    xt = sXT[:, t_abs * BL : (t_abs + 1) * BL]
    for j in range(JC):
        nc.tensor.matmul(
            G[32 * j : 32 * (j + 1), :],
            xt,
            sWU[:, j * GF : (j + 1) * GF],
            start=True,
            stop=skip_main,
            tile_position=(0, 32 * j),
            skip_group_check=True,
        )
    if not skip_main:
        # recurrent term: accumulate h @ Whh^T (folded), 4 col-tiles per k-chunk
        for k in range(KC):
            lhsT = ring[:, prev_slab * P + 32 * k : prev_slab * P + 32 * (k + 1)]
            for j in range(JC):
                nc.tensor.matmul(
                    G[32 * j : 32 * (j + 1), :],
                    lhsT,
                    sW[k][:, j * GF : (j + 1) * GF],
                    start=False,
                    stop=(k == KC - 1),
                    tile_position=(0, 32 * j),
                    skip_group_check=True,
                )

    # activations: fold order along free is [i | f | o | g] x 128.
    # sig(i,f) -> A[:, 0:256] fp32; sig(o) -> so_t bf16 (transposed later);
    # tanh(g) -> A[:, 384:512] fp32.
    A = apool.tile([P, GF], F32, tag="A")
    nc.scalar.activation(A[:, 0 : 2 * P], G[:, 0 : 2 * P], AF.Sigmoid)
    if first_step:
        # c_prev = 0: c = sig(i) * tanh(g)
        nc.scalar.activation(A[:, 3 * P : 4 * P], G[:, 3 * P : 4 * P], AF.Tanh)
        nc.vector.tensor_mul(c_tile, A[:, 0:P], A[:, 3 * P : 4 * P])
    else:
        # c = sig(f) * c  (overlaps the tanh(g) ACT op below)
        nc.vector.tensor_mul(c_tile, A[:, P : 2 * P], c_tile)
        nc.scalar.activation(A[:, 3 * P : 4 * P], G[:, 3 * P : 4 * P], AF.Tanh)
        tmp = spool.tile([P, P], F32, tag="tmp")
        nc.vector.tensor_mul(tmp, A[:, 0:P], A[:, 3 * P : 4 * P])
        nc.vector.tensor_add(c_tile, c_tile, tmp)

    if tap_A is not None:
        nc.vector.tensor_copy(tap_A, G)

    # h^T = sig(o)^T * tanh(c)^T, built directly in transposed space:
    # transpose the two factors (PE) and multiply straight into the ring slab.
    so_t = spool.tile([P, P], mmdt, tag="so")
    nc.scalar.activation(so_t, G[:, 2 * P : 3 * P], AF.Sigmoid)
    T2 = tpool.tile([P, 2 * P], mmdt, tag="T2")
    nc.tensor.transpose(T2[:, 0:P], so_t, ident)
    # sig(o)^T to SBUF early (off the critical chain; DVE reads only one PSUM
    # operand per instruction, so the final mul needs this factor in SBUF)
    soT = spool.tile([P, P], mmdt, tag="soT")
    nc.vector.tensor_copy(soT, T2[:, 0:P])
    tct = spool.tile([P, P], mmdt, tag="tct")
    nc.scalar.activation(tct, c_tile, AF.Tanh)
    nc.tensor.transpose(T2[:, P : 2 * P], tct, ident)
    # chunk 0 first so the next step's k=0 matmul can begin immediately
    slab = ring[:, cur_slab * P : (cur_slab + 1) * P]
    nc.vector.tensor_mul(slab[:, 0:32], soT[:, 0:32], T2[:, P : P + 32])
    nc.vector.tensor_mul(slab[:, 32:P], soT[:, 32:P], T2[:, P + 32 : 2 * P])


def _ys_window(nc, pools, consts, w, dY, nsteps=WIN):
    """Apply Linear to the h^T slabs of decoder window w and DMA the ys out."""
    ypool, ysb_pool = pools["y"], pools["ysb"]
    ring5 = consts["ring5"]  # ring viewed [P, 2, WIN, KC, BL]
    sLW, sLB = consts["LW"], consts["LB"]
    half = w % 2
    yps = ypool.tile([1, WIN * BL], F32, tag="yps")
    for k in range(KC):
        nc.tensor.matmul(
            yps[0:1, 0 : nsteps * BL],
            sLW[:, k : k + 1],
            ring5[:, half, 0:nsteps, k, :],
            start=(k == 0),
            stop=(k == KC - 1),
        )
    ysb = ysb_pool.tile([1, WIN * BL], F32, tag="ysb")
    nc.scalar.activation(
        ysb[0:1, 0 : nsteps * BL], yps[0:1, 0 : nsteps * BL], AF.Identity,
        bias=sLB[0:1, 0:1],
    )
    nc.sync.dma_start(
        dY[0:1, w * WIN * BL : w * WIN * BL + nsteps * BL],
        ysb[0:1, 0 : nsteps * BL],
    )


def build_nc(t_enc=T, t_dec=T, mm_dtype="bf16", debug_taps=False):
    mmdt = _MMDT[mm_dtype]
    # Bacc (not plain Bass): its compile() legalizes semaphore waits (>1 wait per
    # instruction gets split into EventSemaphore chains) — walrus requires it.
    nc = bacc.Bacc()
    if debug_taps:
        dDBG = nc.declare_dram_parameter("DBG", [P, 4 * P], F32, isOutput=True)
        dDBG2 = nc.declare_dram_parameter("DBG2", [P, GF], F32, isOutput=True)

    # XT/UE/UD are K-padded to 128 rows (rows 2.. are zero) so every matmul uses
    # the regular [128, 32] LDW+MM encoding (the 32x32 tile encoding has only one
    # sync-wait slot and walrus dies with "Too many sync wait commands").
    dXT = nc.declare_dram_parameter("XT", [P, max(t_enc, t_dec) * BL], mmdt, isOutput=False)
    dWE = nc.declare_dram_parameter("WE", [KC, P, 4 * GF], mmdt, isOutput=False)
    dWD = nc.declare_dram_parameter("WD", [KC, P, 4 * GF], mmdt, isOutput=False)
    dUE = nc.declare_dram_parameter("UE", [P, 4 * GF], mmdt, isOutput=False)
    dUD = nc.declare_dram_parameter("UD", [P, 4 * GF], mmdt, isOutput=False)
    dLW = nc.declare_dram_parameter("LW", [P, KC], mmdt, isOutput=False)
    dLB = nc.declare_dram_parameter("LB", [1, 1], F32, isOutput=False)
    if mmdt == F32R:
        dID = nc.declare_dram_parameter("ID", [P, P], F32R, isOutput=False)
    dY = nc.declare_dram_parameter("Y", [1, t_dec * BL], F32, isOutput=True)

    with ExitStack() as ctx:
        tc = ctx.enter_context(tile.TileContext(nc))
        const = ctx.enter_context(tc.tile_pool(name="const", bufs=1))
        gpool = ctx.enter_context(tc.tile_pool(name="g", bufs=3, space="PSUM"))
        tpool = ctx.enter_context(tc.tile_pool(name="tps", bufs=2, space="PSUM"))
        ypool = ctx.enter_context(tc.tile_pool(name="yps", bufs=1, space="PSUM"))
        apool = ctx.enter_context(tc.tile_pool(name="act", bufs=3))
        spool = ctx.enter_context(tc.tile_pool(name="small", bufs=3))
        ysb_pool = ctx.enter_context(tc.tile_pool(name="ysb", bufs=2))

        # persistent SBUF tensors
        sXT = const.tile([P, max(t_enc, t_dec) * BL], mmdt, tag="sXT")
        sWE = [
            const.tile([P, 4 * GF], mmdt, tag=f"sWE{k}", name=f"sWE{k}")
            for k in range(KC)
        ]
        sWD = [
            const.tile([P, 4 * GF], mmdt, tag=f"sWD{k}", name=f"sWD{k}")
            for k in range(KC)
        ]
        sUE = const.tile([P, 4 * GF], mmdt, tag="sUE")
        sUD = const.tile([P, 4 * GF], mmdt, tag="sUD")
        sLW = const.tile([P, KC], mmdt, tag="sLW")
        sLB = const.tile([1, 1], F32, tag="sLB")
        # float32r tiles can't be written by memset/affine_select (walrus demands
        # fp32r-rounded producers), so the fp32r identity comes from the host.
        ident = const.tile([P, P], mmdt, tag="ident")
        ring = const.tile([P, RING * P], mmdt, tag="ring")
        c_tile = const.tile([P, P], F32, tag="c")

        nc.sync.dma_start(sXT[:, :], dXT[:, :])
        for k in range(KC):
            nc.sync.dma_start(sWE[k][:, :], dWE[k])
            nc.sync.dma_start(sWD[k][:, :], dWD[k])
        nc.sync.dma_start(sUE[:, :], dUE[:, :])
        nc.sync.dma_start(sUD[:, :], dUD[:, :])
        nc.sync.dma_start(sLW[:, :], dLW[:, :])
        nc.sync.dma_start(sLB[:, :], dLB[:, :])
        if mmdt == F32R:
            nc.sync.dma_start(ident[:, :], dID[:, :])
        else:
            make_identity(nc, ident)

        ring5 = ring.rearrange("p (u s k b) -> p u s k b", u=2, s=WIN, k=KC)
        pools = {
            "g": gpool,
            "t": tpool,
            "a": apool,
            "s": spool,
            "y": ypool,
            "ysb": ysb_pool,
        }
        consts = {
            "ring": ring,
            "ring5": ring5,
            "XT": sXT,
            "ident": ident,
            "LW": sLW,
            "LB": sLB,
        }

        # ---------------- encoder ----------------
        dbg2_sb = None
        if debug_taps:
            dbg2_sb = const.tile([P, GF], F32, tag="dbg2_sb")
        for t in range(t_enc):
            _step(
                nc,
                pools,
                consts,
                t,
                (t - 1) % RING,
                t % RING,
                sWE,
                sUE,
                c_tile,
                first_step=(t == 0),
                skip_main=(t == 0),
                mmdt=mmdt,
                tap_A=(dbg2_sb if (debug_taps and t == 1) else None),
            )
        if debug_taps:
            nc.sync.dma_start(dDBG2[:, :], dbg2_sb)

        if debug_taps:
            # dump h^T slabs of enc steps 0..3 and the act tile of enc step 0
            dbg_sb = const.tile([P, 4 * P], F32, tag="dbg_sb")
            for s in range(4):
                nc.vector.tensor_copy(
                    dbg_sb[:, s * P : (s + 1) * P],
                    ring[:, s * P : (s + 1) * P],
                )
            nc.sync.dma_start(dDBG[:, :], dbg_sb)

        # ---------------- decoder ----------------
        enc_final = (t_enc - 1) % RING
        for t in range(t_dec):
            prev = enc_final if t == 0 else (t - 1) % RING
            _step(
                nc,
                pools,
                consts,
                t,
                prev,
                t % RING,
                sWD,
                sUD,
                c_tile,
                first_step=(t == 0),
                skip_main=False,
                mmdt=mmdt,
            )
            if t % WIN == WIN - 1:
                _ys_window(nc, pools, consts, t // WIN, dY)
            elif t == t_dec - 1:
                _ys_window(nc, pools, consts, t // WIN, dY, nsteps=(t % WIN) + 1)

    if not nc.is_finalized():
        nc.finalize()  # runs Bacc.compile() (wait legalization, reg alloc, ...)
    return nc


def prep_core_inputs(x_core, weights, mm_dtype="bf16"):
    """Host-side layout prep for one core. x_core: [BL, T, 1] fp32."""
    npdt = ml_dtypes.bfloat16 if mm_dtype == "bf16" else np.float32  # fp32r stores fp32 bits
    perm = _perm_fold()
    out = {}
    xt = np.zeros((P, T * BL), dtype=np.float32)
    xt[0] = x_core[:, :, 0].T.reshape(-1)  # t-major: idx = t*BL + b
    xt[1] = 1.0
    out["XT"] = xt.astype(npdt)
    for tag, Wih, Whh, bih, bhh in (
        ("E", weights["enc_Wih"], weights["enc_Whh"], weights["enc_bih"], weights["enc_bhh"]),
        ("D", weights["dec_Wih"], weights["dec_Whh"], weights["dec_bih"], weights["dec_bhh"]),
    ):
        Wf = np.ascontiguousarray(Whh[perm, :].T)  # [H, 4H] folded
        # reshape (C-order) splits H into k-chunks -> [k, p, n]
        out["W" + tag] = Wf.reshape(KC, P, 4 * GF).astype(npdt)
        u = np.zeros((P, 4 * GF), dtype=np.float32)
        u[0] = Wih[perm, 0]
        u[1] = (bih + bhh)[perm]
        out["U" + tag] = u.astype(npdt)
    out["LW"] = np.ascontiguousarray(weights["lin_W"][0].reshape(KC, P).T).astype(npdt)
    out["LB"] = weights["lin_b"].reshape(1, 1).astype(np.float32)
    if mm_dtype == "fp32r":
        out["ID"] = np.eye(P, dtype=np.float32)
    return out


_CACHE = {}
_LAST_RESULTS = None  # BassKernelResults of the most recent run (for profiling)


def bench(inputs, iters=5, mm_dtype="bf16"):
    """Time device execution only: inputs live on device, repeated jit calls.

    Returns (best_seconds, per_iter_seconds). Mirrors bass2jax.run_bass_via_pjrt's
    shard_map construction; only the donated zero output buffers are recreated per
    iteration (they are tiny).
    """
    import time

    import jax
    from jax.experimental.shard_map import shard_map
    from jax.sharding import Mesh, PartitionSpec

    from concourse import bass2jax, mybir as mb

    key = ("full", mm_dtype)
    if key not in _CACHE:
        _CACHE[key] = build_nc(T, T, mm_dtype)
    nc = _CACHE[key]

    x = np.asarray(inputs["x"], dtype=np.float32)
    in_maps = [
        prep_core_inputs(x[i * BL : (i + 1) * BL], inputs, mm_dtype)
        for i in range(N_CORES)
    ]

    bass2jax.install_neuronx_cc_hook()
    in_names, out_names, out_avals, zero_outs = [], [], [], []
    for alloc in nc.m.functions[0].allocations:
        if not isinstance(mb.MemoryLocationSet, type) or not isinstance(
            alloc, mb.MemoryLocationSet
        ):
            continue
        name = alloc.memorylocations[0].name
        part_name = nc.partition_id_tensor.name if nc.partition_id_tensor else None
        if alloc.kind == "ExternalInput":
            if name != part_name:
                in_names.append(name)
        elif alloc.kind == "ExternalOutput":
            out_names.append(name)
            shape = tuple(alloc.tensor_shape)
            dtype = mb.dt.np(alloc.dtype)
            out_avals.append(jax.core.ShapedArray(shape, dtype))
            zero_outs.append(np.zeros(shape, dtype))
    n_params = len(in_names)
    n_outs = len(out_avals)
    part_name = nc.partition_id_tensor.name if nc.partition_id_tensor else None
    all_names = in_names + out_names + ([part_name] if part_name else [])

    def _body(*args):
        operands = list(args)
        if part_name:
            operands.append(bass2jax.partition_id_tensor())
        outs = bass2jax._bass_exec_p.bind(
            *operands,
            out_avals=tuple(out_avals),
            in_names=tuple(all_names),
            out_names=tuple(out_names),
            lowering_input_output_aliases=(),
            sim_require_finite=True,
            sim_require_nnan=True,
            nc=nc,
        )
        return tuple(outs)

    devices = jax.devices()[:N_CORES]
    mesh = Mesh(np.asarray(devices), ("core",))
    donate = tuple(range(n_params, n_params + n_outs))
    sharded = jax.jit(
        shard_map(
            _body,
            mesh=mesh,
            in_specs=(PartitionSpec("core"),) * (n_params + n_outs),
            out_specs=(PartitionSpec("core"),) * n_outs,
            check_rep=False,
        ),
        donate_argnums=donate,
        keep_unused=True,
    )
    concat_in = [
        np.concatenate([np.asarray(in_maps[c][nm]) for c in range(N_CORES)], axis=0)
        for nm in in_names
    ]
    dev_in = jax.device_put(
        concat_in,
        [jax.sharding.NamedSharding(mesh, PartitionSpec("core"))] * n_params,
    )

    def _zeros():
        return [
            np.zeros((N_CORES * z.shape[0], *z.shape[1:]), z.dtype) for z in zero_outs
        ]

    out = sharded(*dev_in, *_zeros())  # warmup/compile
    jax.block_until_ready(out)
    times = []
    for _ in range(iters):
        zs = jax.device_put(
            _zeros(), [jax.sharding.NamedSharding(mesh, PartitionSpec("core"))] * n_outs
        )
        jax.block_until_ready(zs)
        t0 = time.perf_counter()
        out = sharded(*dev_in, *zs)
        jax.block_until_ready(out)
        times.append(time.perf_counter() - t0)
    return min(times), times


def kernel(**inputs) -> np.ndarray:
    global _LAST_RESULTS
    mm_dtype = "bf16"
    key = ("full", mm_dtype)
    if key not in _CACHE:
        _CACHE[key] = build_nc(T, T, mm_dtype)
    nc = _CACHE[key]

    x = np.asarray(inputs["x"], dtype=np.float32)
    in_maps = []
    for i in range(N_CORES):
        x_core = x[i * BL : (i + 1) * BL]
        in_maps.append(prep_core_inputs(x_core, inputs, mm_dtype))

    res = run_bass_kernel_spmd(nc, in_maps, core_ids=list(range(N_CORES)))
    _LAST_RESULTS = res
    y = np.empty((B, T, 1), dtype=np.float32)
    for i in range(N_CORES):
        yi = np.asarray(res.results[i]["Y"], dtype=np.float32).reshape(T, BL)
        y[i * BL : (i + 1) * BL, :, 0] = yi.T
    return y

